# revision 22
# baseline (speedup 1.0000x reference)
"""AugLUT Trainium2 kernel: per-batch random 20-knot LUT applied to x via
piecewise-linear interpolation (out = lerp of normalized ran_y at t = 19x).

Two concurrent chunk pipelines share the 27 chunks per core:

A) DVE clamp-pair chain. With t = 19x,
       f(t) = sum_{k=-1}^{18} D_k * clamp(t - k, 0, 1),
   D_{-1} = y_0, D_k = y_{k+1} - y_k. Two consecutive terms fuse into ONE
   custom DVE instruction (8 ALU stages) via
       clamp(e-1,0,1) = clamp(e,0,2) - clamp(e,0,1)
   so the 20-term LUT costs 10 line-rate DVE ops (+1 ACT scale).

B) relu-basis + PE chunks: f = A + Bt + sum_j c_j relu(t-j). The affine
   tensor and (18-n_d) relu tensors come from ACT (scale/bias folds 19x-j);
   n_d relu tensors come from DVE stock tensor_scalar on an fp16 t, which
   engages the DVE 2x perf mode (~0.51 us per [128,1024] op, measured).
   The PE accumulates all 19 fp16 terms (runtime diagonal fp16 weights,
   512-wide matmuls at 1 cyc/row) in PSUM; ACT copies PSUM->SBUF.

Measured on HW (reps-delta): DVE pair op ~2.32 us per [128,2048]; ACT op
~1.04 us per [128,1024]; Pool/GPSIMD elementwise is ~10 us per [128,512]
(21x worse than its cost model) so Pool does no compute; PSUM cannot be
DMA'd or read by GPSIMD, so copies are on ACT. The split n_pe/n_d
balances DVE (~23 us per A-chunk, ~1 us per DVE-basis op pair) against
ACT (~22 us per PE-chunk) and PE (~20 us per PE-chunk).

Out-DMAs are triggered from the otherwise-idle Pool engine's DGE: an
out-trigger waits on its chunk's compute, and on the shared SP queue that
wait head-of-line-blocks the next chunks' input prefetch (in-DMA triggers
queued behind it), collapsing pipeline depth. Splitting trigger engines
(inputs on SP, outputs on Pool) keeps input prefetch running ahead.

Sharding: pure data parallel - batch b -> NeuronCore b (8 cores); the tiny
LUT/coefficient tensors ride along as per-partition-broadcast inputs.
"""

import sys

if "/opt/trn_rl_repo" not in sys.path:
    sys.path.insert(0, "/opt/trn_rl_repo")

import numpy as np

import concourse.bacc as bacc
import concourse.dve_ops as dve_ops_mod
import concourse.mybir as mybir
from concourse import bass_utils
from concourse.dve_ops import DveOp
from concourse.dve_spec import (
    C0,
    C1,
    C2,
    Latch,
    One,
    Spec,
    Src0,
    Src1,
    Zero,
    lower,
    maxx,
    minn,
    _has_src1,
)
from concourse.dve_uop import DveOpSpec
from concourse.tile import TileContext

N_BINS = 20
EPS = 1e-5
BATCH = 8
SPATIAL = (192, 192, 192)
N_ELEM = 192 * 192 * 192  # 7_077_888
P = 128
FREE = N_ELEM // P  # 55296
CHUNK = 2048
N_CHUNKS = FREE // CHUNK  # 27
SUB = 1024  # basis / PSUM / matmul granularity
N_TERMS = 20  # ones, t, relu(t-1..18)
N_BLOCKS = 21  # weight blocks: identity, A-diag, B-diag, c_1..c_18 diags


# --------------------------------------------------------------------------
# Custom DVE op registration (runtime, self-signed sha) - A-path pair ops
# --------------------------------------------------------------------------
def _pair_body(with_acc: bool):
    e = Src0 - C2
    r = maxx(e, Zero)
    c1 = minn(r, One)
    p1 = c1 * C0
    c2 = minn(r, One + One)
    if with_acc:
        a = Src1 + p1
        p2 = c2 * Latch(maxx(C1, C1))
        return a + p2
    p2 = c2 * C1
    return p1 + p2


def _np_pair(in0, in1, s0, s1, imm2, with_acc):
    e = in0.astype(np.float32) - np.float32(imm2)
    c1 = np.minimum(np.maximum(e, np.float32(0)), np.float32(1))
    c2 = np.minimum(np.maximum(e, np.float32(0)), np.float32(2))
    s0 = np.asarray(s0, dtype=np.float32)
    s1 = np.asarray(s1, dtype=np.float32)
    r = c1 * s0 + c2 * s1
    if with_acc:
        r = r + in1
    return r.astype(np.float32)


def _register(name: str, spec: Spec) -> DveOp:
    for op in dve_ops_mod.OPS:
        if op.name == name:
            return op
    row = dve_ops_mod._CUSTOM_DVE_ROW_BASE + len(dve_ops_mod.OPS)
    assert row < 0x20, "custom-DVE row overflow"
    sha = {}
    for ver in ("v3", "v4"):
        try:
            s = DveOpSpec(
                name=name,
                opcode=row,
                uops=lower(spec, ver=ver),
                rd1_en=_has_src1(spec),
            )
            sha[ver] = s.sha(ver)
        except Exception:
            pass
    op = DveOp(name, spec, subdim=False, uops_sha=sha)
    dve_ops_mod.OPS.append(op)
    dve_ops_mod.CUSTOM_DVE_SPECS[name] = spec
    dve_ops_mod._SUB_OPCODE_FOR_NAME[name] = row
    return op


AUGLUT_PAIR = _register(
    "AUGLUT_PAIR",
    Spec(
        body=_pair_body(with_acc=True),
        reference=lambda in0, in1, s0, s1, imm2: _np_pair(in0, in1, s0, s1, imm2, True),
    ),
)

AUGLUT_PAIR_INIT = _register(
    "AUGLUT_PAIR_INIT",
    Spec(
        body=_pair_body(with_acc=False),
        reference=lambda in0, in1, s0, s1, imm2: _np_pair(
            in0, None, s0, s1, imm2, False
        ),
    ),
)


# --------------------------------------------------------------------------
# Chunk schedule: interleave A / C / B types evenly across the 27 chunks
# --------------------------------------------------------------------------
def _chunk_types(n_chunks: int, n_c: int, n_b: int) -> list:
    """Return a list of 'a'/'c'/'b' of length n_chunks, types spread evenly."""
    assert n_c + n_b <= n_chunks
    types = ["a"] * n_chunks
    used = set()
    if n_b:
        for i in range(n_b):
            pos = int(round((i + 0.5) * n_chunks / n_b - 0.5)) % n_chunks
            while pos in used:
                pos = (pos + 1) % n_chunks
            used.add(pos)
            types[pos] = "b"
    if n_c:
        for i in range(n_c):
            pos = int(round((i + 0.25) * n_chunks / n_c)) % n_chunks
            while pos in used:
                pos = (pos + 1) % n_chunks
            used.add(pos)
            types[pos] = "c"
    return types


# --------------------------------------------------------------------------
# Bass module
# --------------------------------------------------------------------------
def build_module(
    reps: int = 1,
    chunk: int = CHUNK,
    bufs: int = 3,
    n_pe: int = 16,
    n_d: int = 10,
    mm_width: int = 512,
    out_dma: str = "sync",
):
    """Build the SPMD Bass module.

    `reps` repeats the whole compute (HW-time measurement via deltas).
    Of FREE//chunk chunks, n_pe use the basis+PE pipeline (n_d of the 18
    relu basis tensors generated on DVE via fp16 tensor_scalar, the rest
    plus the affine tensor on ACT), and the rest use the DVE clamp-pair
    chain. Pool/GPSIMD does no compute: its software ops are ~8x slower
    on HW than the cost model claims.
    """
    nc = bacc.Bacc("TRN2", target_bir_lowering=False, debug=False, num_devices=BATCH)

    f32 = mybir.dt.float32
    f16 = mybir.dt.float16
    x_d = nc.dram_tensor("x", [P, FREE], f32, kind="ExternalInput")
    lut_d = nc.dram_tensor("lut", [P, N_BINS], f32, kind="ExternalInput")
    use_pe = n_pe > 0
    if use_pe:
        wts_d = nc.dram_tensor("wts", [P, N_BLOCKS * P], f16, kind="ExternalInput")
        ab_d = nc.dram_tensor("ab", [P, 20], f32, kind="ExternalInput")
    o_d = nc.dram_tensor("o", [P, FREE], f32, kind="ExternalOutput")

    x_ap = x_d.ap()
    o_ap = o_d.ap()
    n_chunks = FREE // chunk
    assert n_chunks * chunk == FREE, (chunk, FREE)
    n_sub = chunk // SUB
    n_mm = SUB // mm_width
    types = _chunk_types(n_chunks, n_pe, 0)

    with TileContext(nc) as tc:
        with (
            tc.tile_pool(name="lutp", bufs=1) as lutp,
            tc.tile_pool(name="work", bufs=bufs) as wp,
            tc.tile_pool(name="bas", bufs=2) as bp,
            tc.tile_pool(name="outp", bufs=bufs) as op_,
            tc.tile_pool(name="psum", bufs=4, space="PSUM") as pp,
        ):
            lut_t = lutp.tile([P, N_BINS], f32)
            nc.sync.dma_start(out=lut_t[:], in_=lut_d.ap()[:])
            if use_pe:
                wts_t = lutp.tile([P, N_BLOCKS * P], f16)
                nc.sync.dma_start(out=wts_t[:], in_=wts_d.ap()[:])
                ab_t = lutp.tile([P, 20], f32)
                nc.sync.dma_start(out=ab_t[:], in_=ab_d.ap()[:])
                ones_t = lutp.tile([P, SUB], f16)
                nc.vector.memset(ones_t[:], 1.0)

            def blk(i):
                return wts_t[:, i * P : (i + 1) * P]

            def dve_chunk(sl):
                xt = wp.tile([P, chunk], f32, tag="x")
                nc.sync.dma_start(out=xt[:], in_=x_ap[:, sl])
                nc.scalar.mul(out=xt[:], in_=xt[:], mul=19.0)
                acc = wp.tile([P, chunk], f32, tag="acc")
                nc.vector._custom_dve(
                    AUGLUT_PAIR_INIT,
                    out=acc[:],
                    in0=xt[:],
                    s0=lut_t[:, 0:1],
                    s1=lut_t[:, 1:2],
                    imm2=-1.0,
                )
                for pr in range(1, 10):
                    nc.vector._custom_dve(
                        AUGLUT_PAIR,
                        out=acc[:],
                        in0=xt[:],
                        in1=acc[:],
                        s0=lut_t[:, 2 * pr : 2 * pr + 1],
                        s1=lut_t[:, 2 * pr + 1 : 2 * pr + 2],
                        imm2=float(2 * pr - 1),
                    )
                dma = nc.gpsimd if out_dma == "gpsimd" else nc.sync
                dma.dma_start(out=o_ap[:, sl], in_=acc[:])

            def pe_half(terms, ps_tag="ps"):
                """terms: list of (weight block idx, basis AP [P, SUB])."""
                ps = pp.tile([P, SUB], f32, tag=ps_tag)
                for ti, (bi, bap) in enumerate(terms):
                    for m in range(n_mm):
                        ms = slice(m * mm_width, (m + 1) * mm_width)
                        nc.tensor.matmul(
                            ps[:, ms],
                            blk(bi),
                            bap[:, ms],
                            start=(ti == 0),
                            stop=(ti == len(terms) - 1),
                        )
                return ps

            def finish_chunk(ot, sl):
                dma = nc.gpsimd if out_dma == "gpsimd" else nc.sync
                dma.dma_start(out=o_ap[:, sl], in_=ot[:])

            def pe_chunk(sl):
                xt = wp.tile([P, chunk], f32, tag="x")
                nc.sync.dma_start(out=xt[:], in_=x_ap[:, sl])
                dve_js = set(range(1, 1 + n_d))
                if n_d > 0:
                    tf = wp.tile([P, chunk], f16, tag="t16")
                    nc.scalar.mul(out=tf[:], in_=xt[:], mul=19.0)
                ot = op_.tile([P, chunk], f32, tag="o")
                ps_list = []
                for h in range(n_sub):
                    hs = slice(h * SUB, (h + 1) * SUB)
                    aff = bp.tile([P, SUB], f16, tag="aff")
                    # affine term: A + 19B*x
                    nc.scalar.activation(
                        out=aff[:],
                        in_=xt[:, hs],
                        func=mybir.ActivationFunctionType.Identity,
                        bias=ab_t[:, 0:1],
                        scale=ab_t[:, 1:2],
                    )
                    terms = [(0, aff[:])]
                    for j in range(1, 19):
                        r = bp.tile([P, SUB], f16, tag=f"r{j}")
                        if j in dve_js:
                            # relu basis from t = 19x (fp16, DVE 4x mode)
                            nc.vector.tensor_scalar(
                                r[:],
                                tf[:, hs],
                                float(j),
                                0.0,
                                mybir.AluOpType.subtract,
                                mybir.AluOpType.max,
                            )
                        else:
                            # relu basis: relu(19x - j)
                            nc.scalar.activation(
                                out=r[:],
                                in_=xt[:, hs],
                                func=mybir.ActivationFunctionType.Relu,
                                bias=ab_t[:, 1 + j : 2 + j],
                                scale=19.0,
                            )
                        terms.append((2 + j, r[:]))
                    ps_list.append(pe_half(terms))
                for h, ps in enumerate(ps_list):
                    hs = slice(h * SUB, (h + 1) * SUB)
                    nc.scalar.copy(out=ot[:, hs], in_=ps[:])
                finish_chunk(ot, sl)

            def body():
                for j in range(n_chunks):
                    sl = slice(j * chunk, (j + 1) * chunk)
                    if types[j] == "c":
                        pe_chunk(sl)
                    else:
                        dve_chunk(sl)

            if reps == 1:
                body()
            else:
                with tc.For_i(
                    0,
                    reps,
                    1,
                    hint_engines=(
                        mybir.EngineType.DVE,
                        mybir.EngineType.SP,
                        mybir.EngineType.Activation,
                        mybir.EngineType.PE,
                        mybir.EngineType.Pool,
                    ),
                ):
                    body()

    nc.finalize()
    return nc


_MODULE_CACHE: dict[tuple, object] = {}


def _get_module(reps: int = 1, **cfg):
    key = (reps, tuple(sorted(cfg.items())))
    if key not in _MODULE_CACHE:
        _MODULE_CACHE[key] = build_module(reps, **cfg)
    return _MODULE_CACHE[key]


# --------------------------------------------------------------------------
# Host-side LUT prep
# --------------------------------------------------------------------------
def _make_luts(ran_y: np.ndarray):
    """ran_y [8, 20] -> (lut [8,128,20], wts [8,128,21*128] f16, ab [8,128,20])."""
    y = ran_y.astype(np.float32)
    ymin = y.min(axis=1, keepdims=True)
    ymax = y.max(axis=1, keepdims=True)
    y = (y - ymin) / (ymax - ymin + np.float32(EPS))

    D = np.empty((BATCH, N_BINS), np.float32)
    D[:, 0] = y[:, 0]
    D[:, 1:] = y[:, 1:] - y[:, :-1]

    cols = np.empty((BATCH, N_BINS), np.float32)
    cols[:, 0::2] = D[:, 0::2] - D[:, 1::2]  # s0 of each pair
    cols[:, 1::2] = D[:, 1::2]  # s1 of each pair
    lut = np.broadcast_to(cols[:, None, :], (BATCH, P, N_BINS)).copy()

    # relu-basis: f(t) = A + B*t + sum_{j=1..18} c_j*relu(t-j);  t = 19x
    A = y[:, 0]  # [8]
    B = y[:, 1] - y[:, 0]
    c = (y[:, 2:] - y[:, 1:-1]) - (y[:, 1:-1] - y[:, :-2])  # [8, 18]

    # weight blocks (fp16): 0 identity, 1 A-diag, 2 B-diag, 3.. c_j diags
    wts = np.zeros((BATCH, P, N_BLOCKS * P), np.float16)
    di = np.arange(P)
    wts[:, di, di] = 1.0
    wts[:, di, P + di] = A[:, None].astype(np.float16)
    wts[:, di, 2 * P + di] = B[:, None].astype(np.float16)
    for j in range(1, 19):
        wts[:, di, (2 + j) * P + di] = c[:, j - 1][:, None].astype(np.float16)

    ab = np.empty((BATCH, P, 20), np.float32)
    ab[:, :, 0] = A[:, None]
    ab[:, :, 1] = (np.float32(19.0) * B)[:, None]
    ab[:, :, 2:] = -np.arange(1, 19, dtype=np.float32)[None, None, :]
    return lut, wts, ab


# --------------------------------------------------------------------------
# Entry point
# --------------------------------------------------------------------------
CFG = dict(n_pe=16, n_d=10, bufs=3, out_dma="gpsimd")


def kernel(x: np.ndarray, ran_y: np.ndarray, _reps: int = 1, **_cfg) -> np.ndarray:
    x = np.asarray(x, dtype=np.float32)
    ran_y = np.asarray(ran_y, dtype=np.float32)
    assert x.shape == (BATCH, *SPATIAL), x.shape
    assert ran_y.shape == (BATCH, N_BINS), ran_y.shape

    cfg = {**CFG, **_cfg}
    nc = _get_module(_reps, **cfg)
    lut, wts, ab = _make_luts(ran_y)
    xr = np.ascontiguousarray(x.reshape(BATCH, P, FREE))
    in_maps = []
    for b in range(BATCH):
        m = {"x": xr[b], "lut": lut[b]}
        if cfg.get("n_pe", 0) > 0:
            m["wts"] = wts[b]
            m["ab"] = ab[b]
        in_maps.append(m)

    res = bass_utils.run_bass_kernel_spmd(nc, in_maps, core_ids=list(range(BATCH)))
    out = np.stack([res.results[b]["o"] for b in range(BATCH)], axis=0)
    return out.reshape(BATCH, *SPATIAL)


# revision 23
# speedup vs baseline: 1.0091x; 1.0091x over previous
"""AugLUT Trainium2 kernel: per-batch random 20-knot LUT applied to x via
piecewise-linear interpolation (out = lerp of normalized ran_y at t = 19x).

Two concurrent chunk pipelines share the 27 chunks per core:

A) DVE clamp-pair chain. With t = 19x,
       f(t) = sum_{k=-1}^{18} D_k * clamp(t - k, 0, 1),
   D_{-1} = y_0, D_k = y_{k+1} - y_k. Two consecutive terms fuse into ONE
   custom DVE instruction (8 ALU stages) via
       clamp(e-1,0,1) = clamp(e,0,2) - clamp(e,0,1)
   so the 20-term LUT costs 10 line-rate DVE ops (+1 ACT scale).

B) relu-basis + PE chunks: f = A + Bt + sum_j c_j relu(t-j). The affine
   tensor and (18-n_d) relu tensors come from ACT (scale/bias folds 19x-j);
   n_d relu tensors come from DVE stock tensor_scalar on an fp16 t, which
   engages the DVE 2x perf mode (~0.51 us per [128,1024] op, measured).
   The PE accumulates all 19 fp16 terms (runtime diagonal fp16 weights,
   512-wide matmuls at 1 cyc/row) in PSUM; ACT copies PSUM->SBUF.

Measured on HW (reps-delta): DVE pair op ~2.32 us per [128,2048]; ACT op
~1.04 us per [128,1024]; Pool/GPSIMD elementwise is ~10 us per [128,512]
(21x worse than its cost model) so Pool does no compute; PSUM cannot be
DMA'd or read by GPSIMD, so copies are on ACT. The split n_pe/n_d
balances DVE (~23 us per A-chunk, ~1 us per DVE-basis op pair) against
ACT (~22 us per PE-chunk) and PE (~20 us per PE-chunk).

Out-DMAs are triggered from the otherwise-idle Pool engine's DGE: an
out-trigger waits on its chunk's compute, and on the shared SP queue that
wait head-of-line-blocks the next chunks' input prefetch (in-DMA triggers
queued behind it), collapsing pipeline depth. Splitting trigger engines
(inputs on SP, outputs on Pool) keeps input prefetch running ahead.

Sharding: pure data parallel - batch b -> NeuronCore b (8 cores); the tiny
LUT/coefficient tensors ride along as per-partition-broadcast inputs.
"""

import sys

if "/opt/trn_rl_repo" not in sys.path:
    sys.path.insert(0, "/opt/trn_rl_repo")

import numpy as np

import concourse.bacc as bacc
import concourse.dve_ops as dve_ops_mod
import concourse.mybir as mybir
from concourse import bass_utils
from concourse.dve_ops import DveOp
from concourse.dve_spec import (
    C0,
    C1,
    C2,
    Latch,
    One,
    Spec,
    Src0,
    Src1,
    Zero,
    lower,
    maxx,
    minn,
    _has_src1,
)
from concourse.dve_uop import DveOpSpec
from concourse.tile import TileContext

N_BINS = 20
EPS = 1e-5
BATCH = 8
SPATIAL = (192, 192, 192)
N_ELEM = 192 * 192 * 192  # 7_077_888
P = 128
FREE = N_ELEM // P  # 55296
CHUNK = 2048
N_CHUNKS = FREE // CHUNK  # 27
SUB = 1024  # basis / PSUM / matmul granularity
N_TERMS = 20  # ones, t, relu(t-1..18)
N_BLOCKS = 21  # weight blocks: identity, A-diag, B-diag, c_1..c_18 diags


# --------------------------------------------------------------------------
# Custom DVE op registration (runtime, self-signed sha) - A-path pair ops
# --------------------------------------------------------------------------
def _pair_body(with_acc: bool):
    e = Src0 - C2
    r = maxx(e, Zero)
    c1 = minn(r, One)
    p1 = c1 * C0
    c2 = minn(r, One + One)
    if with_acc:
        a = Src1 + p1
        p2 = c2 * Latch(maxx(C1, C1))
        return a + p2
    p2 = c2 * C1
    return p1 + p2


def _np_pair(in0, in1, s0, s1, imm2, with_acc):
    e = in0.astype(np.float32) - np.float32(imm2)
    c1 = np.minimum(np.maximum(e, np.float32(0)), np.float32(1))
    c2 = np.minimum(np.maximum(e, np.float32(0)), np.float32(2))
    s0 = np.asarray(s0, dtype=np.float32)
    s1 = np.asarray(s1, dtype=np.float32)
    r = c1 * s0 + c2 * s1
    if with_acc:
        r = r + in1
    return r.astype(np.float32)


def _register(name: str, spec: Spec) -> DveOp:
    for op in dve_ops_mod.OPS:
        if op.name == name:
            return op
    row = dve_ops_mod._CUSTOM_DVE_ROW_BASE + len(dve_ops_mod.OPS)
    assert row < 0x20, "custom-DVE row overflow"
    sha = {}
    for ver in ("v3", "v4"):
        try:
            s = DveOpSpec(
                name=name,
                opcode=row,
                uops=lower(spec, ver=ver),
                rd1_en=_has_src1(spec),
            )
            sha[ver] = s.sha(ver)
        except Exception:
            pass
    op = DveOp(name, spec, subdim=False, uops_sha=sha)
    dve_ops_mod.OPS.append(op)
    dve_ops_mod.CUSTOM_DVE_SPECS[name] = spec
    dve_ops_mod._SUB_OPCODE_FOR_NAME[name] = row
    return op


AUGLUT_PAIR = _register(
    "AUGLUT_PAIR",
    Spec(
        body=_pair_body(with_acc=True),
        reference=lambda in0, in1, s0, s1, imm2: _np_pair(in0, in1, s0, s1, imm2, True),
    ),
)

AUGLUT_PAIR_INIT = _register(
    "AUGLUT_PAIR_INIT",
    Spec(
        body=_pair_body(with_acc=False),
        reference=lambda in0, in1, s0, s1, imm2: _np_pair(
            in0, None, s0, s1, imm2, False
        ),
    ),
)


# --------------------------------------------------------------------------
# Chunk schedule: interleave A / C / B types evenly across the 27 chunks
# --------------------------------------------------------------------------
def _chunk_types(n_chunks: int, n_c: int, n_b: int) -> list:
    """Return a list of 'a'/'c'/'b' of length n_chunks, types spread evenly."""
    assert n_c + n_b <= n_chunks
    types = ["a"] * n_chunks
    used = set()
    if n_b:
        for i in range(n_b):
            pos = int(round((i + 0.5) * n_chunks / n_b - 0.5)) % n_chunks
            while pos in used:
                pos = (pos + 1) % n_chunks
            used.add(pos)
            types[pos] = "b"
    if n_c:
        for i in range(n_c):
            pos = int(round((i + 0.25) * n_chunks / n_c)) % n_chunks
            while pos in used:
                pos = (pos + 1) % n_chunks
            used.add(pos)
            types[pos] = "c"
    return types


# --------------------------------------------------------------------------
# Bass module
# --------------------------------------------------------------------------
def build_module(
    reps: int = 1,
    chunk: int = CHUNK,
    bufs: int = 3,
    n_pe: int = 16,
    n_d: int = 10,
    mm_width: int = 512,
    out_dma: str = "sync",
):
    """Build the SPMD Bass module.

    `reps` repeats the whole compute (HW-time measurement via deltas).
    Of FREE//chunk chunks, n_pe use the basis+PE pipeline (n_d of the 18
    relu basis tensors generated on DVE via fp16 tensor_scalar, the rest
    plus the affine tensor on ACT), and the rest use the DVE clamp-pair
    chain. Pool/GPSIMD does no compute: its software ops are ~8x slower
    on HW than the cost model claims.
    """
    nc = bacc.Bacc("TRN2", target_bir_lowering=False, debug=False, num_devices=BATCH)

    f32 = mybir.dt.float32
    f16 = mybir.dt.float16
    x_d = nc.dram_tensor("x", [P, FREE], f32, kind="ExternalInput")
    lut_d = nc.dram_tensor("lut", [P, N_BINS], f32, kind="ExternalInput")
    use_pe = n_pe > 0
    if use_pe:
        wts_d = nc.dram_tensor("wts", [P, N_BLOCKS * P], f16, kind="ExternalInput")
        ab_d = nc.dram_tensor("ab", [P, 20], f32, kind="ExternalInput")
    o_d = nc.dram_tensor("o", [P, FREE], f32, kind="ExternalOutput")

    x_ap = x_d.ap()
    o_ap = o_d.ap()
    n_chunks = FREE // chunk
    assert n_chunks * chunk == FREE, (chunk, FREE)
    n_sub = chunk // SUB
    n_mm = SUB // mm_width
    types = _chunk_types(n_chunks, n_pe, 0)

    with TileContext(nc) as tc:
        with (
            tc.tile_pool(name="lutp", bufs=1) as lutp,
            tc.tile_pool(name="work", bufs=bufs) as wp,
            tc.tile_pool(name="bas", bufs=2) as bp,
            tc.tile_pool(name="outp", bufs=bufs) as op_,
            tc.tile_pool(name="psum", bufs=4, space="PSUM") as pp,
        ):
            lut_t = lutp.tile([P, N_BINS], f32)
            nc.sync.dma_start(out=lut_t[:], in_=lut_d.ap()[:])
            if use_pe:
                wts_t = lutp.tile([P, N_BLOCKS * P], f16)
                nc.sync.dma_start(out=wts_t[:], in_=wts_d.ap()[:])
                ab_t = lutp.tile([P, 20], f32)
                nc.sync.dma_start(out=ab_t[:], in_=ab_d.ap()[:])
                ones_t = lutp.tile([P, SUB], f16)
                nc.vector.memset(ones_t[:], 1.0)

            def blk(i):
                return wts_t[:, i * P : (i + 1) * P]

            def dve_chunk(sl):
                xt = wp.tile([P, chunk], f32, tag="x")
                nc.sync.dma_start(out=xt[:], in_=x_ap[:, sl])
                nc.scalar.mul(out=xt[:], in_=xt[:], mul=19.0)
                acc = wp.tile([P, chunk], f32, tag="acc")
                # two interleaved half-chains hide the chained ops'
                # write-ack dependency stall on the DVE
                halves = [slice(0, chunk // 2), slice(chunk // 2, chunk)]
                for ch in halves:
                    nc.vector._custom_dve(
                        AUGLUT_PAIR_INIT,
                        out=acc[:, ch],
                        in0=xt[:, ch],
                        s0=lut_t[:, 0:1],
                        s1=lut_t[:, 1:2],
                        imm2=-1.0,
                    )
                for pr in range(1, 10):
                    for ch in halves:
                        nc.vector._custom_dve(
                            AUGLUT_PAIR,
                            out=acc[:, ch],
                            in0=xt[:, ch],
                            in1=acc[:, ch],
                            s0=lut_t[:, 2 * pr : 2 * pr + 1],
                            s1=lut_t[:, 2 * pr + 1 : 2 * pr + 2],
                            imm2=float(2 * pr - 1),
                        )
                dma = nc.gpsimd if out_dma == "gpsimd" else nc.sync
                dma.dma_start(out=o_ap[:, sl], in_=acc[:])

            def pe_half(terms, ps_tag="ps"):
                """terms: list of (weight block idx, basis AP [P, SUB])."""
                ps = pp.tile([P, SUB], f32, tag=ps_tag)
                for ti, (bi, bap) in enumerate(terms):
                    for m in range(n_mm):
                        ms = slice(m * mm_width, (m + 1) * mm_width)
                        nc.tensor.matmul(
                            ps[:, ms],
                            blk(bi),
                            bap[:, ms],
                            start=(ti == 0),
                            stop=(ti == len(terms) - 1),
                        )
                return ps

            def finish_chunk(ot, sl):
                dma = nc.gpsimd if out_dma == "gpsimd" else nc.sync
                dma.dma_start(out=o_ap[:, sl], in_=ot[:])

            def pe_chunk(sl):
                xt = wp.tile([P, chunk], f32, tag="x")
                nc.sync.dma_start(out=xt[:], in_=x_ap[:, sl])
                dve_js = set(range(1, 1 + n_d))
                if n_d > 0:
                    tf = wp.tile([P, chunk], f16, tag="t16")
                    nc.scalar.mul(out=tf[:], in_=xt[:], mul=19.0)
                ot = op_.tile([P, chunk], f32, tag="o")
                ps_list = []
                for h in range(n_sub):
                    hs = slice(h * SUB, (h + 1) * SUB)
                    aff = bp.tile([P, SUB], f16, tag="aff")
                    # affine term: A + 19B*x
                    nc.scalar.activation(
                        out=aff[:],
                        in_=xt[:, hs],
                        func=mybir.ActivationFunctionType.Identity,
                        bias=ab_t[:, 0:1],
                        scale=ab_t[:, 1:2],
                    )
                    terms = [(0, aff[:])]
                    for j in range(1, 19):
                        r = bp.tile([P, SUB], f16, tag=f"r{j}")
                        if j in dve_js:
                            # relu basis from t = 19x (fp16, DVE 4x mode)
                            nc.vector.tensor_scalar(
                                r[:],
                                tf[:, hs],
                                float(j),
                                0.0,
                                mybir.AluOpType.subtract,
                                mybir.AluOpType.max,
                            )
                        else:
                            # relu basis: relu(19x - j)
                            nc.scalar.activation(
                                out=r[:],
                                in_=xt[:, hs],
                                func=mybir.ActivationFunctionType.Relu,
                                bias=ab_t[:, 1 + j : 2 + j],
                                scale=19.0,
                            )
                        terms.append((2 + j, r[:]))
                    ps_list.append(pe_half(terms))
                for h, ps in enumerate(ps_list):
                    hs = slice(h * SUB, (h + 1) * SUB)
                    nc.scalar.copy(out=ot[:, hs], in_=ps[:])
                finish_chunk(ot, sl)

            def body():
                for j in range(n_chunks):
                    sl = slice(j * chunk, (j + 1) * chunk)
                    if types[j] == "c":
                        pe_chunk(sl)
                    else:
                        dve_chunk(sl)

            if reps == 1:
                body()
            else:
                with tc.For_i(
                    0,
                    reps,
                    1,
                    hint_engines=(
                        mybir.EngineType.DVE,
                        mybir.EngineType.SP,
                        mybir.EngineType.Activation,
                        mybir.EngineType.PE,
                        mybir.EngineType.Pool,
                    ),
                ):
                    body()

    nc.finalize()
    return nc


_MODULE_CACHE: dict[tuple, object] = {}


def _get_module(reps: int = 1, **cfg):
    key = (reps, tuple(sorted(cfg.items())))
    if key not in _MODULE_CACHE:
        _MODULE_CACHE[key] = build_module(reps, **cfg)
    return _MODULE_CACHE[key]


# --------------------------------------------------------------------------
# Host-side LUT prep
# --------------------------------------------------------------------------
def _make_luts(ran_y: np.ndarray):
    """ran_y [8, 20] -> (lut [8,128,20], wts [8,128,21*128] f16, ab [8,128,20])."""
    y = ran_y.astype(np.float32)
    ymin = y.min(axis=1, keepdims=True)
    ymax = y.max(axis=1, keepdims=True)
    y = (y - ymin) / (ymax - ymin + np.float32(EPS))

    D = np.empty((BATCH, N_BINS), np.float32)
    D[:, 0] = y[:, 0]
    D[:, 1:] = y[:, 1:] - y[:, :-1]

    cols = np.empty((BATCH, N_BINS), np.float32)
    cols[:, 0::2] = D[:, 0::2] - D[:, 1::2]  # s0 of each pair
    cols[:, 1::2] = D[:, 1::2]  # s1 of each pair
    lut = np.broadcast_to(cols[:, None, :], (BATCH, P, N_BINS)).copy()

    # relu-basis: f(t) = A + B*t + sum_{j=1..18} c_j*relu(t-j);  t = 19x
    A = y[:, 0]  # [8]
    B = y[:, 1] - y[:, 0]
    c = (y[:, 2:] - y[:, 1:-1]) - (y[:, 1:-1] - y[:, :-2])  # [8, 18]

    # weight blocks (fp16): 0 identity, 1 A-diag, 2 B-diag, 3.. c_j diags
    wts = np.zeros((BATCH, P, N_BLOCKS * P), np.float16)
    di = np.arange(P)
    wts[:, di, di] = 1.0
    wts[:, di, P + di] = A[:, None].astype(np.float16)
    wts[:, di, 2 * P + di] = B[:, None].astype(np.float16)
    for j in range(1, 19):
        wts[:, di, (2 + j) * P + di] = c[:, j - 1][:, None].astype(np.float16)

    ab = np.empty((BATCH, P, 20), np.float32)
    ab[:, :, 0] = A[:, None]
    ab[:, :, 1] = (np.float32(19.0) * B)[:, None]
    ab[:, :, 2:] = -np.arange(1, 19, dtype=np.float32)[None, None, :]
    return lut, wts, ab


# --------------------------------------------------------------------------
# Entry point
# --------------------------------------------------------------------------
CFG = dict(n_pe=16, n_d=10, bufs=3, out_dma="gpsimd")


def kernel(x: np.ndarray, ran_y: np.ndarray, _reps: int = 1, **_cfg) -> np.ndarray:
    x = np.asarray(x, dtype=np.float32)
    ran_y = np.asarray(ran_y, dtype=np.float32)
    assert x.shape == (BATCH, *SPATIAL), x.shape
    assert ran_y.shape == (BATCH, N_BINS), ran_y.shape

    cfg = {**CFG, **_cfg}
    nc = _get_module(_reps, **cfg)
    lut, wts, ab = _make_luts(ran_y)
    xr = np.ascontiguousarray(x.reshape(BATCH, P, FREE))
    in_maps = []
    for b in range(BATCH):
        m = {"x": xr[b], "lut": lut[b]}
        if cfg.get("n_pe", 0) > 0:
            m["wts"] = wts[b]
            m["ab"] = ab[b]
        in_maps.append(m)

    res = bass_utils.run_bass_kernel_spmd(nc, in_maps, core_ids=list(range(BATCH)))
    out = np.stack([res.results[b]["o"] for b in range(BATCH)], axis=0)
    return out.reshape(BATCH, *SPATIAL)
